# revision 30
# baseline (speedup 1.0000x reference)
"""Trainium2 Bass kernel for nn_CrossAttentionFusion (GNN message passing).

Sharding: data-parallel over target nodes (8 cores x 2500 targets).
Per core: a combined fp16 K/V table for BOTH layers is built on-device once
(K/V depend only on spatial_embed), then each 128-target block gathers its
padded neighbor rows ONCE (2KB/row covers both layers) and runs both
transformer layers back-to-back in SBUF. Targets are degree-sorted on host
so each block uses a tight per-block K. PE GEMMs run fp16 with fp32 PSUM
accumulation; softmax/LayerNorm run fp32 on DVE/ACT.
"""

import numpy as np
from contextlib import ExitStack

import concourse.bass as bass
import concourse.bacc as bacc
import concourse.tile as tile
import concourse.mybir as mybir
from concourse import bass_utils

N = 20000
D = 256
H = 4
DH = 64
L = 2
E = 320000
KCAP = 48
NCORES = 8
NS = N // NCORES          # 2500 targets per core
NBLK = 20                 # 128-target blocks per core
TPAD = NBLK * 128         # 2560
NPAD = 157 * 128          # 20096 node-table rows (padded)
EPS = 1e-5
MASKVAL = -30000.0        # pre-scale additive mask; *0.125 -> exp underflows to 0
SCALE = 1.0 / np.sqrt(DH)

f32 = mybir.dt.float32
f16 = mybir.dt.float16

_prog_cache = {}


def _build_neighbors(edge_index):
    """Mirror of reference._build_neighbors in numpy. Returns nbr, slots."""
    src = edge_index[0].astype(np.int64)
    tgt = edge_index[1].astype(np.int64)
    counts = np.bincount(tgt, minlength=N).astype(np.int64)
    order = np.argsort(tgt, kind="stable")
    src_s, tgt_s = src[order], tgt[order]
    offsets = np.concatenate([[0], np.cumsum(counts)[:-1]])
    pos = np.arange(E, dtype=np.int64) - offsets[tgt_s]
    keep = pos < KCAP
    nbr = np.zeros((N, KCAP), np.int32)
    nbr[tgt_s[keep], pos[keep]] = src_s[keep]
    slots = np.minimum(counts, KCAP).astype(np.int32)
    iso = counts == 0
    nbr[iso, 0] = np.nonzero(iso)[0]
    slots[iso] = 1
    return nbr, slots


def _host_prep(inputs):
    edge_index = np.asarray(inputs["edge_index"]).astype(np.int64)
    nbr, slots = _build_neighbors(edge_index)

    per_core = []
    for c in range(NCORES):
        ids = np.arange(c * NS, (c + 1) * NS)
        order = np.argsort(slots[ids], kind="stable")
        ids_sorted = ids[order]
        ndum = TPAD - NS
        per_core.append(
            np.concatenate([np.full(ndum, -1, np.int64), ids_sorted]))

    # per-block K shared across cores (SPMD: one program)
    kb = np.zeros(NBLK, np.int64)
    for c in range(NCORES):
        tg = per_core[c]
        s = np.where(tg >= 0, slots[np.clip(tg, 0, N - 1)], 1)
        for b in range(NBLK):
            kb[b] = max(kb[b], s[b * 128:(b + 1) * 128].max())
    kblocks = tuple(int(min(KCAP, -(-k // 4) * 4)) for k in kb)

    expr = np.asarray(inputs["expr_embed"], np.float32)
    in_maps = []
    tgt_ids = []
    for c in range(NCORES):
        tg = per_core[c]
        valid = tg >= 0
        tgc = np.clip(tg, 0, N - 1)
        s = np.where(valid, slots[tgc], 1)
        nb = nbr[tgc]
        nb[~valid] = 0
        x0 = np.where(valid[:, None], expr[tgc], 0.0).astype(np.float32)

        idx_cols, mask_cols = [], []
        for b in range(NBLK):
            K = kblocks[b]
            bn = nb[b * 128:(b + 1) * 128, :K]
            bs = s[b * 128:(b + 1) * 128]
            validsl = np.arange(K)[None, :] < bs[:, None]
            idx_cols.append(np.where(validsl, bn, 0).astype(np.int32))
            mask_cols.append(
                np.where(validsl, 0.0, MASKVAL).astype(np.float32))
        in_maps.append({
            "x0": x0,
            "idxs": np.ascontiguousarray(np.concatenate(idx_cols, axis=1)),
            "masks": np.ascontiguousarray(np.concatenate(mask_cols, axis=1)),
        })
        tgt_ids.append(tg)

    ipw = np.asarray(inputs["in_proj_w"], np.float32)
    ipb = np.asarray(inputs["in_proj_b"], np.float32)
    opw = np.asarray(inputs["out_proj_w"], np.float32)
    opb = np.asarray(inputs["out_proj_b"], np.float32)
    w1 = np.asarray(inputs["ffn_w1"], np.float32)
    b1 = np.asarray(inputs["ffn_b1"], np.float32)
    w2 = np.asarray(inputs["ffn_w2"], np.float32)
    b2 = np.asarray(inputs["ffn_b2"], np.float32)

    h16 = np.float16
    shared = {
        "spatialT": np.ascontiguousarray(
            np.pad(np.asarray(inputs["spatial_embed"], np.float32),
                   ((0, NPAD - N), (0, 0))).T).astype(h16),
        "wqT": np.ascontiguousarray(ipw[:, :D, :].transpose(0, 2, 1)).astype(h16),
        "wkvT": np.ascontiguousarray(ipw[:, D:, :].transpose(0, 2, 1)).astype(h16),
        "woT": np.ascontiguousarray(opw.transpose(0, 2, 1)).astype(h16),
        "w1T": np.ascontiguousarray(w1.transpose(0, 2, 1)).astype(h16),
        "w2T": np.ascontiguousarray(w2.transpose(0, 2, 1)).astype(h16),
        "bq": ipb[:, :D].reshape(L, 1, D).astype(h16),
        "bkv": ipb[:, D:].reshape(L, 1, 2 * D).astype(h16),
        "bo": opb.reshape(L, 1, D).astype(h16),
        "b1": b1.reshape(L, 1, 2 * D).astype(h16),
        "b2": b2.reshape(L, 1, D).astype(h16),
        "ln1g": np.asarray(inputs["ln1_g"], np.float32).reshape(L, 1, D),
        "ln1b": np.asarray(inputs["ln1_b"], np.float32).reshape(L, 1, D),
        "ln2g": np.asarray(inputs["ln2_g"], np.float32).reshape(L, 1, D),
        "ln2b": np.asarray(inputs["ln2_b"], np.float32).reshape(L, 1, D),
        "ident32": np.eye(128, dtype=np.float32),
        "ident16": np.eye(128, dtype=h16),
        "ones16": np.ones((1, 128), h16),
    }
    for m in in_maps:
        m.update(shared)
    return in_maps, tgt_ids, kblocks


def _build_program(kblocks):
    nc = bacc.Bacc("TRN2", target_bir_lowering=False, debug=False,
                   num_devices=NCORES)
    MW = sum(kblocks)

    dts = {
        "x0": ((TPAD, D), f32), "idxs": ((128, MW), mybir.dt.int32),
        "masks": ((128, MW), f32),
        "spatialT": ((D, NPAD), f16),
        "wqT": ((L, D, D), f16), "wkvT": ((L, D, 2 * D), f16),
        "woT": ((L, D, D), f16), "w1T": ((L, D, 2 * D), f16),
        "w2T": ((L, 2 * D, D), f16),
        "bq": ((L, 1, D), f16), "bkv": ((L, 1, 2 * D), f16),
        "bo": ((L, 1, D), f16), "b1": ((L, 1, 2 * D), f16),
        "b2": ((L, 1, D), f16),
        "ln1g": ((L, 1, D), f32), "ln1b": ((L, 1, D), f32),
        "ln2g": ((L, 1, D), f32), "ln2b": ((L, 1, D), f32),
        "ident32": ((128, 128), f32), "ident16": ((128, 128), f16),
        "ones16": ((1, 128), f16),
    }
    dr = {k: nc.dram_tensor(k, sh, dt, kind="ExternalInput")
          for k, (sh, dt) in dts.items()}
    out_dram = nc.dram_tensor("out", (TPAD, D), f32, kind="ExternalOutput")

    with tile.TileContext(nc) as tc, ExitStack() as ctx:
        ep = ctx.enter_context
        const_p = ep(tc.tile_pool(name="const", bufs=1))
        kvd = ep(tc.tile_pool(name="kvd", bufs=1, space="DRAM"))

        ident32 = const_p.tile([128, 128], f32)
        nc.sync.dma_start(ident32[:], dr["ident32"].ap())
        ident16 = const_p.tile([128, 128], f16)
        nc.sync.dma_start(ident16[:], dr["ident16"].ap())
        ones16 = const_p.tile([1, 128], f16)
        nc.sync.dma_start(ones16[:], dr["ones16"].ap())
        idx_sb = const_p.tile([128, MW], mybir.dt.int32)
        nc.sync.dma_start(idx_sb[:], dr["idxs"].ap())
        mask_sb = const_p.tile([128, MW], f32)
        nc.sync.dma_start(mask_sb[:], dr["masks"].ap())

        def ldw(name, chunks, ncol):
            t = const_p.tile([128, L, chunks, ncol], f16, tag="w_" + name)
            nc.sync.dma_start(
                t[:], dr[name].ap().rearrange("l (c p) n -> p l c n", p=128))
            return t
        wq_sb = ldw("wqT", 2, D)
        wkv_sb = ldw("wkvT", 2, 2 * D)
        wo_sb = ldw("woT", 2, D)
        w1_sb = ldw("w1T", 2, 2 * D)
        w2_sb = ldw("w2T", 4, D)
        brow = {}
        for name, ncol in (("bq", D), ("bkv", 2 * D), ("bo", D),
                           ("b1", 2 * D), ("b2", D)):
            t = const_p.tile([1, L, ncol], f16, tag="b_" + name)
            nc.sync.dma_start(t[:], dr[name].ap().rearrange("l o n -> o l n"))
            brow[name] = t
        lnbc = {}
        for name in ("ln1g", "ln1b", "ln2g", "ln2b"):
            t = const_p.tile([128, L, D], f32, tag="ln_" + name)
            nc.sync.dma_start(
                t[:], dr[name].ap().rearrange("l o n -> o l n")
                .broadcast_to([128, L, D]))
            lnbc[name] = t
        eps_sb = const_p.tile([128, 1], f32)
        nc.vector.memset(eps_sb[:], float(EPS))

        # combined K/V table: row = [k_l0 | v_l0 | k_l1 | v_l1], 2KB fp16
        kvtab = kvd.tile([NPAD, 2 * L * D], f16)

        # ---------- phase 0: K/V tables for both layers ----------
        with tc.tile_pool(name="p0sp", bufs=2) as p0sp, \
             tc.tile_pool(name="p0st", bufs=4) as p0st, \
             tc.tile_pool(name="p0ps", bufs=4, space="PSUM") as p0ps:
            CH = 4096
            off = 0
            while off < NPAD:
                w = min(CH, NPAD - off)
                sp0 = p0sp.tile([128, w], f16, tag="sp0")
                nc.sync.dma_start(sp0[:], dr["spatialT"].ap()[0:128, off:off + w])
                sp1 = p0sp.tile([128, w], f16, tag="sp1")
                nc.sync.dma_start(sp1[:], dr["spatialT"].ap()[128:256, off:off + w])
                for blk in range(w // 128):
                    st = p0st.tile([128, 2 * L * D], f16, tag="kvst")
                    for l in range(L):
                        ps = p0ps.tile([128, 2 * D], f32, tag="kvps")
                        nc.tensor.matmul(ps[:], sp0[:, bass.ts(blk, 128)],
                                         wkv_sb[:, l, 0, :], start=True, stop=False)
                        nc.tensor.matmul(ps[:], sp1[:, bass.ts(blk, 128)],
                                         wkv_sb[:, l, 1, :], start=False, stop=False)
                        nc.tensor.matmul(ps[:], ones16[:], brow["bkv"][:, l, :],
                                         start=False, stop=True)
                        nc.vector.tensor_copy(
                            st[:, 2 * D * l: 2 * D * (l + 1)], ps[:])
                    nc.sync.dma_start(
                        kvtab[off + blk * 128: off + (blk + 1) * 128, :], st[:])
                off += w

        # ---------- per-block processing, both layers ----------
        with tc.tile_pool(name="kvg", bufs=1) as kvgp, \
             tc.tile_pool(name="prod", bufs=1) as prodp, \
             tc.tile_pool(name="small", bufs=2) as smallp, \
             tc.tile_pool(name="mid", bufs=2) as midp, \
             tc.tile_pool(name="lnp", bufs=1) as lnp, \
             tc.tile_pool(name="psmm", bufs=3, space="PSUM") as psmm, \
             tc.tile_pool(name="pstp", bufs=4, space="PSUM") as pstp:

            def transpose_to_f16(src_ap, chunks, dst_tag):
                dst = midp.tile([128, chunks, 128], f16, tag=dst_tag)
                ident = ident32 if src_ap.dtype == f32 else ident16
                for cix in range(chunks):
                    tp = pstp.tile([128, 128], src_ap.dtype, tag="tp")
                    nc.tensor.transpose(tp[:], src_ap[:, bass.ts(cix, 128)],
                                        ident[:])
                    nc.vector.tensor_copy(dst[:, cix, :], tp[:])
                return dst

            def layernorm(src_ap, add_psum, gbc, bbc, out_tag):
                xr = lnp.tile([128, D], f32, tag="ln_xr")
                nc.vector.tensor_tensor(xr[:], src_ap, add_psum,
                                        op=mybir.AluOpType.add)
                sm = smallp.tile([128, 1], f32, tag="ln_sm")
                nc.vector.tensor_reduce(sm[:], xr[:],
                                        axis=mybir.AxisListType.X,
                                        op=mybir.AluOpType.add)
                mu = smallp.tile([128, 1], f32, tag="ln_mu")
                nc.vector.tensor_scalar_mul(mu[:], sm[:], 1.0 / D)
                xc = lnp.tile([128, D], f32, tag="ln_xc")
                nc.vector.tensor_scalar(xc[:], xr[:], scalar1=mu[:],
                                        scalar2=None,
                                        op0=mybir.AluOpType.subtract)
                sq = lnp.tile([128, D], f32, tag="ln_sq")
                nc.vector.tensor_tensor(sq[:], xc[:], xc[:],
                                        op=mybir.AluOpType.mult)
                vs = smallp.tile([128, 1], f32, tag="ln_vs")
                nc.vector.tensor_reduce(vs[:], sq[:],
                                        axis=mybir.AxisListType.X,
                                        op=mybir.AluOpType.add)
                var = smallp.tile([128, 1], f32, tag="ln_var")
                nc.vector.tensor_scalar_mul(var[:], vs[:], 1.0 / D)
                sd = smallp.tile([128, 1], f32, tag="ln_sd")
                nc.scalar.activation(sd[:], var[:],
                                     mybir.ActivationFunctionType.Sqrt,
                                     bias=eps_sb[:])
                rstd = smallp.tile([128, 1], f32, tag="ln_rs")
                nc.vector.reciprocal(rstd[:], sd[:])
                t1 = lnp.tile([128, D], f32, tag="ln_t1")
                nc.vector.tensor_scalar(t1[:], xc[:], scalar1=rstd[:],
                                        scalar2=None,
                                        op0=mybir.AluOpType.mult)
                t2 = lnp.tile([128, D], f32, tag="ln_t2")
                nc.vector.tensor_tensor(t2[:], t1[:], gbc,
                                        op=mybir.AluOpType.mult)
                xo = lnp.tile([128, D], f32, tag=out_tag)
                nc.vector.tensor_tensor(xo[:], t2[:], bbc,
                                        op=mybir.AluOpType.add)
                return xo

            moffs = np.concatenate([[0], np.cumsum(kblocks)]).astype(int)

            for b in range(NBLK):
                K = kblocks[b]
                mo = int(moffs[b])
                # one gather covers K and V for BOTH layers (2KB rows)
                kvg = kvgp.tile([128, K, 2 * L * D], f16, tag="kvg")
                for k in range(K):
                    nc.gpsimd.indirect_dma_start(
                        out=kvg[:, k, :], out_offset=None,
                        in_=kvtab[:],
                        in_offset=bass.IndirectOffsetOnAxis(
                            ap=idx_sb[:, mo + k:mo + k + 1], axis=0))

                xblk_t = midp.tile([128, D], f32, tag="xblk")
                nc.sync.dma_start(xblk_t[:],
                                  dr["x0"].ap()[b * 128:(b + 1) * 128, :])
                xcur = xblk_t[:]

                for l in range(L):
                    # q projection
                    xT = transpose_to_f16(xcur, 2, "xT")
                    qp = psmm.tile([128, D], f32, tag="mm")
                    nc.tensor.matmul(qp[:], xT[:, 0, :], wq_sb[:, l, 0, :],
                                     start=True, stop=False)
                    nc.tensor.matmul(qp[:], xT[:, 1, :], wq_sb[:, l, 1, :],
                                     start=False, stop=False)
                    nc.tensor.matmul(qp[:], ones16[:], brow["bq"][:, l, :],
                                     start=False, stop=True)
                    qh = smallp.tile([128, D], f16, tag="qh")
                    nc.vector.tensor_copy(qh[:], qp[:])

                    # scores = sum_d q*k -> [128, H, K] (two head-pairs)
                    k_ap = kvg[:, :, 2 * D * l: 2 * D * l + D].rearrange(
                        "p s (h d) -> p h s d", h=H)
                    q_ap = (qh[:].rearrange("p (h d) -> p h d", h=H)
                            .unsqueeze(2).broadcast_to([128, H, K, DH]))
                    scores = smallp.tile([128, H, K], f32, tag="scores")
                    for hp in range(2):
                        prod = prodp.tile([128, 2, K, DH], f16, tag="prod")
                        nc.vector.tensor_tensor(
                            prod[:], k_ap[:, 2 * hp:2 * hp + 2],
                            q_ap[:, 2 * hp:2 * hp + 2],
                            op=mybir.AluOpType.mult)
                        nc.vector.tensor_reduce(
                            scores[:, 2 * hp:2 * hp + 2, :], prod[:],
                            axis=mybir.AxisListType.X,
                            op=mybir.AluOpType.add)
                    masked = smallp.tile([128, H, K], f32, tag="masked")
                    m_ap = (mask_sb[:, mo:mo + K].unsqueeze(1)
                            .broadcast_to([128, H, K]))
                    nc.vector.tensor_tensor(masked[:], scores[:], m_ap,
                                            op=mybir.AluOpType.add)
                    ex = smallp.tile([128, H, K], f32, tag="ex")
                    nc.scalar.activation(ex[:], masked[:],
                                         mybir.ActivationFunctionType.Exp,
                                         scale=float(SCALE))
                    denom = smallp.tile([128, H], f32, tag="denom")
                    nc.vector.tensor_reduce(denom[:], ex[:],
                                            axis=mybir.AxisListType.X,
                                            op=mybir.AluOpType.add)
                    rden = smallp.tile([128, H], f32, tag="rden")
                    nc.vector.reciprocal(rden[:], denom[:])
                    alpha = smallp.tile([128, H, K], f16, tag="alpha")
                    r_ap = rden[:].unsqueeze(2).broadcast_to([128, H, K])
                    nc.vector.tensor_tensor(alpha[:], ex[:], r_ap,
                                            op=mybir.AluOpType.mult)

                    # AV: sum_s alpha*v -> [128, H, DH]
                    v_ap = kvg[:, :, 2 * D * l + D: 2 * D * (l + 1)].rearrange(
                        "p s (h d) -> p h d s", h=H)
                    a_ap = alpha[:].unsqueeze(2).broadcast_to([128, H, DH, K])
                    ao = smallp.tile([128, H, DH], f32, tag="ao")
                    for hp in range(2):
                        prod2 = prodp.tile([128, 2, DH, K], f16, tag="prod")
                        nc.vector.tensor_tensor(
                            prod2[:], v_ap[:, 2 * hp:2 * hp + 2],
                            a_ap[:, 2 * hp:2 * hp + 2],
                            op=mybir.AluOpType.mult)
                        nc.vector.tensor_reduce(
                            ao[:, 2 * hp:2 * hp + 2, :], prod2[:],
                            axis=mybir.AxisListType.X,
                            op=mybir.AluOpType.add)

                    # out projection
                    aoT = transpose_to_f16(
                        ao[:].rearrange("p h d -> p (h d)"), 2, "aoT")
                    pso = psmm.tile([128, D], f32, tag="mm")
                    nc.tensor.matmul(pso[:], aoT[:, 0, :], wo_sb[:, l, 0, :],
                                     start=True, stop=False)
                    nc.tensor.matmul(pso[:], aoT[:, 1, :], wo_sb[:, l, 1, :],
                                     start=False, stop=False)
                    nc.tensor.matmul(pso[:], ones16[:], brow["bo"][:, l, :],
                                     start=False, stop=True)

                    x1 = layernorm(xcur, pso[:], lnbc["ln1g"][:, l, :],
                                   lnbc["ln1b"][:, l, :], "x1_%d" % l)

                    # FFN
                    x1T = transpose_to_f16(x1[:], 2, "x1T")
                    psh = psmm.tile([128, 2 * D], f32, tag="mm")
                    nc.tensor.matmul(psh[:], x1T[:, 0, :], w1_sb[:, l, 0, :],
                                     start=True, stop=False)
                    nc.tensor.matmul(psh[:], x1T[:, 1, :], w1_sb[:, l, 1, :],
                                     start=False, stop=False)
                    nc.tensor.matmul(psh[:], ones16[:], brow["b1"][:, l, :],
                                     start=False, stop=True)
                    hh = midp.tile([128, 2 * D], f16, tag="hh")
                    nc.scalar.activation(hh[:], psh[:],
                                         mybir.ActivationFunctionType.Gelu)
                    hT = transpose_to_f16(hh[:], 4, "hT")
                    psy = psmm.tile([128, D], f32, tag="mm")
                    for cix in range(4):
                        nc.tensor.matmul(psy[:], hT[:, cix, :],
                                         w2_sb[:, l, cix, :],
                                         start=(cix == 0), stop=False)
                    nc.tensor.matmul(psy[:], ones16[:], brow["b2"][:, l, :],
                                     start=False, stop=True)

                    x2 = layernorm(x1[:], psy[:], lnbc["ln2g"][:, l, :],
                                   lnbc["ln2b"][:, l, :], "x2_%d" % l)
                    xcur = x2[:]

                nc.sync.dma_start(out_dram.ap()[b * 128:(b + 1) * 128, :],
                                  xcur)

    nc.compile()
    return nc


def kernel(**inputs) -> np.ndarray:
    in_maps, tgt_ids, kblocks = _host_prep(inputs)
    if kblocks not in _prog_cache:
        _prog_cache[kblocks] = _build_program(kblocks)
    nc = _prog_cache[kblocks]
    res = bass_utils.run_bass_kernel_spmd(nc, in_maps,
                                          core_ids=list(range(NCORES)))
    out = np.zeros((N, D), np.float32)
    for c in range(NCORES):
        o = res.results[c]["out"]
        tg = tgt_ids[c]
        valid = tg >= 0
        out[tg[valid]] = o[valid]
    return out


# revision 33
# speedup vs baseline: 1.0554x; 1.0554x over previous
"""Trainium2 Bass kernel for nn_CrossAttentionFusion (GNN message passing).

Sharding: data-parallel over target nodes (8 cores x 2500 targets).
Per core: a combined fp16 K/V table for BOTH layers is built on-device once
(K/V depend only on spatial_embed), then each 128-target block gathers its
padded neighbor rows ONCE (2KB/row covers both layers) and runs both
transformer layers back-to-back in SBUF. Targets are degree-sorted on host
so each block uses a tight per-block K. PE GEMMs run fp16 with fp32 PSUM
accumulation; softmax/LayerNorm run fp32 on DVE/ACT.
"""

import numpy as np
from contextlib import ExitStack

import concourse.bass as bass
import concourse.bacc as bacc
import concourse.tile as tile
import concourse.mybir as mybir
from concourse import bass_utils

N = 20000
D = 256
H = 4
DH = 64
L = 2
E = 320000
KCAP = 48
NCORES = 8
NS = N // NCORES          # 2500 targets per core
NBLK = 20                 # 128-target blocks per core
TPAD = NBLK * 128         # 2560
NPAD = 157 * 128          # 20096 node-table rows (padded)
EPS = 1e-5
MASKVAL = -30000.0        # pre-scale additive mask; *0.125 -> exp underflows to 0
SCALE = 1.0 / np.sqrt(DH)

f32 = mybir.dt.float32
f16 = mybir.dt.float16

_prog_cache = {}


def _build_neighbors(edge_index):
    """Mirror of reference._build_neighbors in numpy. Returns nbr, slots."""
    src = edge_index[0].astype(np.int64)
    tgt = edge_index[1].astype(np.int64)
    counts = np.bincount(tgt, minlength=N).astype(np.int64)
    order = np.argsort(tgt, kind="stable")
    src_s, tgt_s = src[order], tgt[order]
    offsets = np.concatenate([[0], np.cumsum(counts)[:-1]])
    pos = np.arange(E, dtype=np.int64) - offsets[tgt_s]
    keep = pos < KCAP
    nbr = np.zeros((N, KCAP), np.int32)
    nbr[tgt_s[keep], pos[keep]] = src_s[keep]
    slots = np.minimum(counts, KCAP).astype(np.int32)
    iso = counts == 0
    nbr[iso, 0] = np.nonzero(iso)[0]
    slots[iso] = 1
    return nbr, slots


def _host_prep(inputs):
    edge_index = np.asarray(inputs["edge_index"]).astype(np.int64)
    nbr, slots = _build_neighbors(edge_index)

    per_core = []
    for c in range(NCORES):
        ids = np.arange(c * NS, (c + 1) * NS)
        order = np.argsort(slots[ids], kind="stable")
        ids_sorted = ids[order]
        ndum = TPAD - NS
        per_core.append(
            np.concatenate([np.full(ndum, -1, np.int64), ids_sorted]))

    # per-block K shared across cores (SPMD: one program)
    kb = np.zeros(NBLK, np.int64)
    for c in range(NCORES):
        tg = per_core[c]
        s = np.where(tg >= 0, slots[np.clip(tg, 0, N - 1)], 1)
        for b in range(NBLK):
            kb[b] = max(kb[b], s[b * 128:(b + 1) * 128].max())
    kblocks = tuple(int(min(KCAP, -(-k // 4) * 4)) for k in kb)

    expr = np.asarray(inputs["expr_embed"], np.float32)
    in_maps = []
    tgt_ids = []
    for c in range(NCORES):
        tg = per_core[c]
        valid = tg >= 0
        tgc = np.clip(tg, 0, N - 1)
        s = np.where(valid, slots[tgc], 1)
        nb = nbr[tgc]
        nb[~valid] = 0
        x0 = np.where(valid[:, None], expr[tgc], 0.0).astype(np.float32)

        idx_cols, mask_cols = [], []
        for b in range(NBLK):
            K = kblocks[b]
            bn = nb[b * 128:(b + 1) * 128, :K]
            bs = s[b * 128:(b + 1) * 128]
            validsl = np.arange(K)[None, :] < bs[:, None]
            idx_cols.append(np.where(validsl, bn, 0).astype(np.int32))
            mask_cols.append(
                np.where(validsl, 0.0, MASKVAL).astype(np.float32))
        in_maps.append({
            "x0": x0,
            "idxs": np.ascontiguousarray(np.concatenate(idx_cols, axis=1)),
            "masks": np.ascontiguousarray(np.concatenate(mask_cols, axis=1)),
        })
        tgt_ids.append(tg)

    ipw = np.asarray(inputs["in_proj_w"], np.float32)
    ipb = np.asarray(inputs["in_proj_b"], np.float32)
    opw = np.asarray(inputs["out_proj_w"], np.float32)
    opb = np.asarray(inputs["out_proj_b"], np.float32)
    w1 = np.asarray(inputs["ffn_w1"], np.float32)
    b1 = np.asarray(inputs["ffn_b1"], np.float32)
    w2 = np.asarray(inputs["ffn_w2"], np.float32)
    b2 = np.asarray(inputs["ffn_b2"], np.float32)

    h16 = np.float16
    shared = {
        "spatialT": np.ascontiguousarray(
            np.pad(np.asarray(inputs["spatial_embed"], np.float32),
                   ((0, NPAD - N), (0, 0))).T).astype(h16),
        "wqT": np.ascontiguousarray(ipw[:, :D, :].transpose(0, 2, 1)).astype(h16),
        "wkvT": np.ascontiguousarray(ipw[:, D:, :].transpose(0, 2, 1)).astype(h16),
        "woT": np.ascontiguousarray(opw.transpose(0, 2, 1)).astype(h16),
        "w1T": np.ascontiguousarray(w1.transpose(0, 2, 1)).astype(h16),
        "w2T": np.ascontiguousarray(w2.transpose(0, 2, 1)).astype(h16),
        "bq": ipb[:, :D].reshape(L, 1, D).astype(h16),
        "bkv": ipb[:, D:].reshape(L, 1, 2 * D).astype(h16),
        "bo": opb.reshape(L, 1, D).astype(h16),
        "b1": b1.reshape(L, 1, 2 * D).astype(h16),
        "b2": b2.reshape(L, 1, D).astype(h16),
        "ln1g": np.asarray(inputs["ln1_g"], np.float32).reshape(L, 1, D),
        "ln1b": np.asarray(inputs["ln1_b"], np.float32).reshape(L, 1, D),
        "ln2g": np.asarray(inputs["ln2_g"], np.float32).reshape(L, 1, D),
        "ln2b": np.asarray(inputs["ln2_b"], np.float32).reshape(L, 1, D),
        "ident32": np.eye(128, dtype=np.float32),
        "ident16": np.eye(128, dtype=h16),
        "ones16": np.ones((1, 128), h16),
    }
    for m in in_maps:
        m.update(shared)
    return in_maps, tgt_ids, kblocks


def _build_program(kblocks):
    nc = bacc.Bacc("TRN2", target_bir_lowering=False, debug=False,
                   num_devices=NCORES)
    MW = sum(kblocks)

    dts = {
        "x0": ((TPAD, D), f32), "idxs": ((128, MW), mybir.dt.int32),
        "masks": ((128, MW), f32),
        "spatialT": ((D, NPAD), f16),
        "wqT": ((L, D, D), f16), "wkvT": ((L, D, 2 * D), f16),
        "woT": ((L, D, D), f16), "w1T": ((L, D, 2 * D), f16),
        "w2T": ((L, 2 * D, D), f16),
        "bq": ((L, 1, D), f16), "bkv": ((L, 1, 2 * D), f16),
        "bo": ((L, 1, D), f16), "b1": ((L, 1, 2 * D), f16),
        "b2": ((L, 1, D), f16),
        "ln1g": ((L, 1, D), f32), "ln1b": ((L, 1, D), f32),
        "ln2g": ((L, 1, D), f32), "ln2b": ((L, 1, D), f32),
        "ident32": ((128, 128), f32), "ident16": ((128, 128), f16),
        "ones16": ((1, 128), f16),
    }
    dr = {k: nc.dram_tensor(k, sh, dt, kind="ExternalInput")
          for k, (sh, dt) in dts.items()}
    out_dram = nc.dram_tensor("out", (TPAD, D), f32, kind="ExternalOutput")

    with tile.TileContext(nc) as tc, ExitStack() as ctx:
        ep = ctx.enter_context
        const_p = ep(tc.tile_pool(name="const", bufs=1))
        kvd = ep(tc.tile_pool(name="kvd", bufs=1, space="DRAM"))

        ident32 = const_p.tile([128, 128], f32)
        nc.sync.dma_start(ident32[:], dr["ident32"].ap())
        ident16 = const_p.tile([128, 128], f16)
        nc.sync.dma_start(ident16[:], dr["ident16"].ap())
        ones16 = const_p.tile([1, 128], f16)
        nc.sync.dma_start(ones16[:], dr["ones16"].ap())
        idx_sb = const_p.tile([128, MW], mybir.dt.int32)
        nc.sync.dma_start(idx_sb[:], dr["idxs"].ap())
        mask_sb = const_p.tile([128, MW], f32)
        nc.sync.dma_start(mask_sb[:], dr["masks"].ap())

        def ldw(name, chunks, ncol):
            t = const_p.tile([128, L, chunks, ncol], f16, tag="w_" + name)
            nc.sync.dma_start(
                t[:], dr[name].ap().rearrange("l (c p) n -> p l c n", p=128))
            return t
        wq_sb = ldw("wqT", 2, D)
        wkv_sb = ldw("wkvT", 2, 2 * D)
        wo_sb = ldw("woT", 2, D)
        w1_sb = ldw("w1T", 2, 2 * D)
        w2_sb = ldw("w2T", 4, D)
        brow = {}
        for name, ncol in (("bq", D), ("bkv", 2 * D), ("bo", D),
                           ("b1", 2 * D), ("b2", D)):
            t = const_p.tile([1, L, ncol], f16, tag="b_" + name)
            nc.sync.dma_start(t[:], dr[name].ap().rearrange("l o n -> o l n"))
            brow[name] = t
        lnbc = {}
        for name in ("ln1g", "ln1b", "ln2g", "ln2b"):
            t = const_p.tile([128, L, D], f32, tag="ln_" + name)
            nc.sync.dma_start(
                t[:], dr[name].ap().rearrange("l o n -> o l n")
                .broadcast_to([128, L, D]))
            lnbc[name] = t
        eps_sb = const_p.tile([128, 1], f32)
        nc.vector.memset(eps_sb[:], float(EPS))

        # combined K/V table: row = [k_l0 | v_l0 | k_l1 | v_l1], 2KB fp16
        kvtab = kvd.tile([NPAD, 2 * L * D], f16)

        # ---------- phase 0: K/V tables for both layers ----------
        with tc.tile_pool(name="p0sp", bufs=2) as p0sp, \
             tc.tile_pool(name="p0st", bufs=4) as p0st, \
             tc.tile_pool(name="p0ps", bufs=4, space="PSUM") as p0ps:
            CH = 4096
            off = 0
            while off < NPAD:
                w = min(CH, NPAD - off)
                sp0 = p0sp.tile([128, w], f16, tag="sp0")
                nc.sync.dma_start(sp0[:], dr["spatialT"].ap()[0:128, off:off + w])
                sp1 = p0sp.tile([128, w], f16, tag="sp1")
                nc.sync.dma_start(sp1[:], dr["spatialT"].ap()[128:256, off:off + w])
                for blk in range(w // 128):
                    st = p0st.tile([128, 2 * L * D], f16, tag="kvst")
                    for l in range(L):
                        ps = p0ps.tile([128, 2 * D], f32, tag="kvps")
                        nc.tensor.matmul(ps[:], sp0[:, bass.ts(blk, 128)],
                                         wkv_sb[:, l, 0, :], start=True, stop=False)
                        nc.tensor.matmul(ps[:], sp1[:, bass.ts(blk, 128)],
                                         wkv_sb[:, l, 1, :], start=False, stop=False)
                        nc.tensor.matmul(ps[:], ones16[:], brow["bkv"][:, l, :],
                                         start=False, stop=True)
                        nc.vector.tensor_copy(
                            st[:, 2 * D * l: 2 * D * (l + 1)], ps[:])
                    nc.sync.dma_start(
                        kvtab[off + blk * 128: off + (blk + 1) * 128, :], st[:])
                off += w

        # ---------- per-block processing, both layers ----------
        with tc.tile_pool(name="prod", bufs=1) as prodp, \
             tc.tile_pool(name="small", bufs=2) as smallp, \
             tc.tile_pool(name="mid", bufs=2) as midp, \
             tc.tile_pool(name="lnp", bufs=1) as lnp, \
             tc.tile_pool(name="psmm", bufs=3, space="PSUM") as psmm, \
             tc.tile_pool(name="pstp", bufs=4, space="PSUM") as pstp:

            def transpose_to_f16(src_ap, chunks, dst_tag):
                dst = midp.tile([128, chunks, 128], f16, tag=dst_tag)
                ident = ident32 if src_ap.dtype == f32 else ident16
                for cix in range(chunks):
                    tp = pstp.tile([128, 128], src_ap.dtype, tag="tp")
                    nc.tensor.transpose(tp[:], src_ap[:, bass.ts(cix, 128)],
                                        ident[:])
                    nc.vector.tensor_copy(dst[:, cix, :], tp[:])
                return dst

            def layernorm(src_ap, add_psum, gbc, bbc, out_tag):
                xr = lnp.tile([128, D], f32, tag="ln_xr")
                nc.vector.tensor_tensor(xr[:], src_ap, add_psum,
                                        op=mybir.AluOpType.add)
                sm = smallp.tile([128, 1], f32, tag="ln_sm")
                nc.vector.tensor_reduce(sm[:], xr[:],
                                        axis=mybir.AxisListType.X,
                                        op=mybir.AluOpType.add)
                mu = smallp.tile([128, 1], f32, tag="ln_mu")
                nc.vector.tensor_scalar_mul(mu[:], sm[:], 1.0 / D)
                xc = lnp.tile([128, D], f32, tag="ln_xc")
                nc.vector.tensor_scalar(xc[:], xr[:], scalar1=mu[:],
                                        scalar2=None,
                                        op0=mybir.AluOpType.subtract)
                sq = lnp.tile([128, D], f32, tag="ln_sq")
                nc.vector.tensor_tensor(sq[:], xc[:], xc[:],
                                        op=mybir.AluOpType.mult)
                vs = smallp.tile([128, 1], f32, tag="ln_vs")
                nc.vector.tensor_reduce(vs[:], sq[:],
                                        axis=mybir.AxisListType.X,
                                        op=mybir.AluOpType.add)
                var = smallp.tile([128, 1], f32, tag="ln_var")
                nc.vector.tensor_scalar_mul(var[:], vs[:], 1.0 / D)
                sd = smallp.tile([128, 1], f32, tag="ln_sd")
                nc.scalar.activation(sd[:], var[:],
                                     mybir.ActivationFunctionType.Sqrt,
                                     bias=eps_sb[:])
                rstd = smallp.tile([128, 1], f32, tag="ln_rs")
                nc.vector.reciprocal(rstd[:], sd[:])
                t1 = lnp.tile([128, D], f32, tag="ln_t1")
                nc.vector.tensor_scalar(t1[:], xc[:], scalar1=rstd[:],
                                        scalar2=None,
                                        op0=mybir.AluOpType.mult)
                t2 = lnp.tile([128, D], f32, tag="ln_t2")
                nc.vector.tensor_tensor(t2[:], t1[:], gbc,
                                        op=mybir.AluOpType.mult)
                xo = lnp.tile([128, D], f32, tag=out_tag)
                nc.vector.tensor_tensor(xo[:], t2[:], bbc,
                                        op=mybir.AluOpType.add)
                return xo

            moffs = np.concatenate([[0], np.cumsum(kblocks)]).astype(int)

            def do_block(b, pool, tag):
                K = kblocks[b]
                mo = int(moffs[b])
                # one gather covers K and V for BOTH layers (2KB rows)
                kvg = pool.tile([128, K, 2 * L * D], f16, tag=tag)
                for k in range(K):
                    nc.gpsimd.indirect_dma_start(
                        out=kvg[:, k, :], out_offset=None,
                        in_=kvtab[:],
                        in_offset=bass.IndirectOffsetOnAxis(
                            ap=idx_sb[:, mo + k:mo + k + 1], axis=0))

                xblk_t = midp.tile([128, D], f32, tag="xblk")
                nc.sync.dma_start(xblk_t[:],
                                  dr["x0"].ap()[b * 128:(b + 1) * 128, :])
                xcur = xblk_t[:]

                for l in range(L):
                    # q projection
                    xT = transpose_to_f16(xcur, 2, "xT")
                    qp = psmm.tile([128, D], f32, tag="mm")
                    nc.tensor.matmul(qp[:], xT[:, 0, :], wq_sb[:, l, 0, :],
                                     start=True, stop=False)
                    nc.tensor.matmul(qp[:], xT[:, 1, :], wq_sb[:, l, 1, :],
                                     start=False, stop=False)
                    nc.tensor.matmul(qp[:], ones16[:], brow["bq"][:, l, :],
                                     start=False, stop=True)
                    qh = smallp.tile([128, D], f16, tag="qh")
                    nc.vector.tensor_copy(qh[:], qp[:])

                    # scores = sum_d q*k -> [128, H, K] (two head-pairs)
                    k_ap = kvg[:, :, 2 * D * l: 2 * D * l + D].rearrange(
                        "p s (h d) -> p h s d", h=H)
                    q_ap = (qh[:].rearrange("p (h d) -> p h d", h=H)
                            .unsqueeze(2).broadcast_to([128, H, K, DH]))
                    scores = smallp.tile([128, H, K], f32, tag="scores")
                    for hp in range(2):
                        prod = prodp.tile([128, 2, K, DH], f16, tag="prod")
                        nc.vector.tensor_tensor(
                            prod[:], k_ap[:, 2 * hp:2 * hp + 2],
                            q_ap[:, 2 * hp:2 * hp + 2],
                            op=mybir.AluOpType.mult)
                        nc.vector.tensor_reduce(
                            scores[:, 2 * hp:2 * hp + 2, :], prod[:],
                            axis=mybir.AxisListType.X,
                            op=mybir.AluOpType.add)
                    masked = smallp.tile([128, H, K], f32, tag="masked")
                    m_ap = (mask_sb[:, mo:mo + K].unsqueeze(1)
                            .broadcast_to([128, H, K]))
                    nc.vector.tensor_tensor(masked[:], scores[:], m_ap,
                                            op=mybir.AluOpType.add)
                    ex = smallp.tile([128, H, K], f32, tag="ex")
                    nc.scalar.activation(ex[:], masked[:],
                                         mybir.ActivationFunctionType.Exp,
                                         scale=float(SCALE))
                    denom = smallp.tile([128, H], f32, tag="denom")
                    nc.vector.tensor_reduce(denom[:], ex[:],
                                            axis=mybir.AxisListType.X,
                                            op=mybir.AluOpType.add)
                    rden = smallp.tile([128, H], f32, tag="rden")
                    nc.vector.reciprocal(rden[:], denom[:])
                    alpha = smallp.tile([128, H, K], f16, tag="alpha")
                    r_ap = rden[:].unsqueeze(2).broadcast_to([128, H, K])
                    nc.vector.tensor_tensor(alpha[:], ex[:], r_ap,
                                            op=mybir.AluOpType.mult)

                    # AV: sum_s alpha*v -> [128, H, DH]
                    v_ap = kvg[:, :, 2 * D * l + D: 2 * D * (l + 1)].rearrange(
                        "p s (h d) -> p h d s", h=H)
                    a_ap = alpha[:].unsqueeze(2).broadcast_to([128, H, DH, K])
                    ao = smallp.tile([128, H, DH], f32, tag="ao")
                    for hp in range(2):
                        prod2 = prodp.tile([128, 2, DH, K], f16, tag="prod")
                        nc.vector.tensor_tensor(
                            prod2[:], v_ap[:, 2 * hp:2 * hp + 2],
                            a_ap[:, 2 * hp:2 * hp + 2],
                            op=mybir.AluOpType.mult)
                        nc.vector.tensor_reduce(
                            ao[:, 2 * hp:2 * hp + 2, :], prod2[:],
                            axis=mybir.AxisListType.X,
                            op=mybir.AluOpType.add)

                    # out projection
                    aoT = transpose_to_f16(
                        ao[:].rearrange("p h d -> p (h d)"), 2, "aoT")
                    pso = psmm.tile([128, D], f32, tag="mm")
                    nc.tensor.matmul(pso[:], aoT[:, 0, :], wo_sb[:, l, 0, :],
                                     start=True, stop=False)
                    nc.tensor.matmul(pso[:], aoT[:, 1, :], wo_sb[:, l, 1, :],
                                     start=False, stop=False)
                    nc.tensor.matmul(pso[:], ones16[:], brow["bo"][:, l, :],
                                     start=False, stop=True)

                    x1 = layernorm(xcur, pso[:], lnbc["ln1g"][:, l, :],
                                   lnbc["ln1b"][:, l, :], "x1_%d" % l)

                    # FFN
                    x1T = transpose_to_f16(x1[:], 2, "x1T")
                    psh = psmm.tile([128, 2 * D], f32, tag="mm")
                    nc.tensor.matmul(psh[:], x1T[:, 0, :], w1_sb[:, l, 0, :],
                                     start=True, stop=False)
                    nc.tensor.matmul(psh[:], x1T[:, 1, :], w1_sb[:, l, 1, :],
                                     start=False, stop=False)
                    nc.tensor.matmul(psh[:], ones16[:], brow["b1"][:, l, :],
                                     start=False, stop=True)
                    hh = midp.tile([128, 2 * D], f16, tag="hh")
                    nc.scalar.activation(hh[:], psh[:],
                                         mybir.ActivationFunctionType.Gelu)
                    hT = transpose_to_f16(hh[:], 4, "hT")
                    psy = psmm.tile([128, D], f32, tag="mm")
                    for cix in range(4):
                        nc.tensor.matmul(psy[:], hT[:, cix, :],
                                         w2_sb[:, l, cix, :],
                                         start=(cix == 0), stop=False)
                    nc.tensor.matmul(psy[:], ones16[:], brow["b2"][:, l, :],
                                     start=False, stop=True)

                    x2 = layernorm(x1[:], psy[:], lnbc["ln2g"][:, l, :],
                                   lnbc["ln2b"][:, l, :], "x2_%d" % l)
                    xcur = x2[:]

                nc.sync.dma_start(out_dram.ap()[b * 128:(b + 1) * 128, :],
                                  xcur)

            # low-K blocks get a double-buffered gather pool (overlap the
            # next block's gather with this block's compute); the high-K
            # tail runs afterward in its own single-buffer scope.
            KSPLIT = 24
            small_blocks = [b for b in range(NBLK) if kblocks[b] <= KSPLIT]
            big_blocks = [b for b in range(NBLK) if kblocks[b] > KSPLIT]
            if small_blocks:
                with tc.tile_pool(name="kvgA", bufs=2) as kvA:
                    for b in small_blocks:
                        do_block(b, kvA, "kvgA")
            if big_blocks:
                with tc.tile_pool(name="kvgB", bufs=1) as kvB:
                    for b in big_blocks:
                        do_block(b, kvB, "kvgB")

    nc.compile()
    return nc


def kernel(**inputs) -> np.ndarray:
    in_maps, tgt_ids, kblocks = _host_prep(inputs)
    if kblocks not in _prog_cache:
        _prog_cache[kblocks] = _build_program(kblocks)
    nc = _prog_cache[kblocks]
    res = bass_utils.run_bass_kernel_spmd(nc, in_maps,
                                          core_ids=list(range(NCORES)))
    out = np.zeros((N, D), np.float32)
    for c in range(NCORES):
        o = res.results[c]["out"]
        tg = tgt_ids[c]
        valid = tg >= 0
        out[tg[valid]] = o[valid]
    return out


# revision 36
# speedup vs baseline: 1.0994x; 1.0417x over previous
"""Trainium2 Bass kernel for nn_CrossAttentionFusion (GNN message passing).

Sharding: data-parallel over target nodes (8 cores x 2500 targets).
Per core: a combined fp16 K/V table for BOTH layers is built on-device once
(K/V depend only on spatial_embed), then each 128-target block gathers its
padded neighbor rows ONCE (2KB/row covers both layers) and runs both
transformer layers back-to-back in SBUF. Targets are degree-sorted on host
so each block uses a tight per-block K. PE GEMMs run fp16 with fp32 PSUM
accumulation; softmax/LayerNorm run fp32 on DVE/ACT.
"""

import numpy as np
from contextlib import ExitStack

import concourse.bass as bass
import concourse.bacc as bacc
import concourse.tile as tile
import concourse.mybir as mybir
from concourse import bass_utils

N = 20000
D = 256
H = 4
DH = 64
L = 2
E = 320000
KCAP = 48
NCORES = 8
NS = N // NCORES          # 2500 targets per core
NBLK = 20                 # 128-target blocks per core
TPAD = NBLK * 128         # 2560
NPAD = 157 * 128          # 20096 node-table rows (padded)
EPS = 1e-5
MASKVAL = -30000.0        # pre-scale additive mask; *0.125 -> exp underflows to 0
SCALE = 1.0 / np.sqrt(DH)

f32 = mybir.dt.float32
f16 = mybir.dt.float16

_prog_cache = {}


def _build_neighbors(edge_index):
    """Mirror of reference._build_neighbors in numpy. Returns nbr, slots."""
    src = edge_index[0].astype(np.int64)
    tgt = edge_index[1].astype(np.int64)
    counts = np.bincount(tgt, minlength=N).astype(np.int64)
    order = np.argsort(tgt, kind="stable")
    src_s, tgt_s = src[order], tgt[order]
    offsets = np.concatenate([[0], np.cumsum(counts)[:-1]])
    pos = np.arange(E, dtype=np.int64) - offsets[tgt_s]
    keep = pos < KCAP
    nbr = np.zeros((N, KCAP), np.int32)
    nbr[tgt_s[keep], pos[keep]] = src_s[keep]
    slots = np.minimum(counts, KCAP).astype(np.int32)
    iso = counts == 0
    nbr[iso, 0] = np.nonzero(iso)[0]
    slots[iso] = 1
    return nbr, slots


def _host_prep(inputs):
    edge_index = np.asarray(inputs["edge_index"]).astype(np.int64)
    nbr, slots = _build_neighbors(edge_index)

    per_core = []
    for c in range(NCORES):
        ids = np.arange(c * NS, (c + 1) * NS)
        order = np.argsort(slots[ids], kind="stable")
        ids_sorted = ids[order]
        ndum = TPAD - NS
        per_core.append(
            np.concatenate([np.full(ndum, -1, np.int64), ids_sorted]))

    # per-block K shared across cores (SPMD: one program)
    kb = np.zeros(NBLK, np.int64)
    for c in range(NCORES):
        tg = per_core[c]
        s = np.where(tg >= 0, slots[np.clip(tg, 0, N - 1)], 1)
        for b in range(NBLK):
            kb[b] = max(kb[b], s[b * 128:(b + 1) * 128].max())
    kblocks = tuple(int(min(KCAP, -(-k // 4) * 4)) for k in kb)

    expr = np.asarray(inputs["expr_embed"], np.float32)
    in_maps = []
    tgt_ids = []
    for c in range(NCORES):
        tg = per_core[c]
        valid = tg >= 0
        tgc = np.clip(tg, 0, N - 1)
        s = np.where(valid, slots[tgc], 1)
        nb = nbr[tgc]
        nb[~valid] = 0
        x0 = np.where(valid[:, None], expr[tgc], 0.0).astype(np.float32)

        idx_cols, mask_cols = [], []
        for b in range(NBLK):
            K = kblocks[b]
            bn = nb[b * 128:(b + 1) * 128, :K]
            bs = s[b * 128:(b + 1) * 128]
            validsl = np.arange(K)[None, :] < bs[:, None]
            idx_cols.append(np.where(validsl, bn, 0).astype(np.int32))
            mask_cols.append(
                np.where(validsl, 0.0, MASKVAL).astype(np.float32))
        in_maps.append({
            "x0": x0,
            "idxs": np.ascontiguousarray(np.concatenate(idx_cols, axis=1)),
            "masks": np.ascontiguousarray(np.concatenate(mask_cols, axis=1)),
        })
        tgt_ids.append(tg)

    ipw = np.asarray(inputs["in_proj_w"], np.float32)
    ipb = np.asarray(inputs["in_proj_b"], np.float32)
    opw = np.asarray(inputs["out_proj_w"], np.float32)
    opb = np.asarray(inputs["out_proj_b"], np.float32)
    w1 = np.asarray(inputs["ffn_w1"], np.float32)
    b1 = np.asarray(inputs["ffn_b1"], np.float32)
    w2 = np.asarray(inputs["ffn_w2"], np.float32)
    b2 = np.asarray(inputs["ffn_b2"], np.float32)

    h16 = np.float16
    shared = {
        "spatialT": np.ascontiguousarray(
            np.pad(np.asarray(inputs["spatial_embed"], np.float32),
                   ((0, NPAD - N), (0, 0))).T).astype(h16),
        "wqT": np.ascontiguousarray(ipw[:, :D, :].transpose(0, 2, 1)).astype(h16),
        "wkvT": np.ascontiguousarray(ipw[:, D:, :].transpose(0, 2, 1)).astype(h16),
        "woT": np.ascontiguousarray(opw.transpose(0, 2, 1)).astype(h16),
        "w1T": np.ascontiguousarray(w1.transpose(0, 2, 1)).astype(h16),
        "w2T": np.ascontiguousarray(w2.transpose(0, 2, 1)).astype(h16),
        "bq": ipb[:, :D].reshape(L, 1, D).astype(h16),
        "bkv": ipb[:, D:].reshape(L, 1, 2 * D).astype(h16),
        "bo": opb.reshape(L, 1, D).astype(h16),
        "b1": b1.reshape(L, 1, 2 * D).astype(h16),
        "b2": b2.reshape(L, 1, D).astype(h16),
        "ln1g": np.asarray(inputs["ln1_g"], np.float32).reshape(L, 1, D),
        "ln1b": np.asarray(inputs["ln1_b"], np.float32).reshape(L, 1, D),
        "ln2g": np.asarray(inputs["ln2_g"], np.float32).reshape(L, 1, D),
        "ln2b": np.asarray(inputs["ln2_b"], np.float32).reshape(L, 1, D),
        "ident32": np.eye(128, dtype=np.float32),
        "ident16": np.eye(128, dtype=h16),
        "ones16": np.ones((1, 128), h16),
    }
    for m in in_maps:
        m.update(shared)
    return in_maps, tgt_ids, kblocks


def _build_program(kblocks):
    nc = bacc.Bacc("TRN2", target_bir_lowering=False, debug=False,
                   num_devices=NCORES)
    MW = sum(kblocks)

    dts = {
        "x0": ((TPAD, D), f32), "idxs": ((128, MW), mybir.dt.int32),
        "masks": ((128, MW), f32),
        "spatialT": ((D, NPAD), f16),
        "wqT": ((L, D, D), f16), "wkvT": ((L, D, 2 * D), f16),
        "woT": ((L, D, D), f16), "w1T": ((L, D, 2 * D), f16),
        "w2T": ((L, 2 * D, D), f16),
        "bq": ((L, 1, D), f16), "bkv": ((L, 1, 2 * D), f16),
        "bo": ((L, 1, D), f16), "b1": ((L, 1, 2 * D), f16),
        "b2": ((L, 1, D), f16),
        "ln1g": ((L, 1, D), f32), "ln1b": ((L, 1, D), f32),
        "ln2g": ((L, 1, D), f32), "ln2b": ((L, 1, D), f32),
        "ident32": ((128, 128), f32), "ident16": ((128, 128), f16),
        "ones16": ((1, 128), f16),
    }
    dr = {k: nc.dram_tensor(k, sh, dt, kind="ExternalInput")
          for k, (sh, dt) in dts.items()}
    out_dram = nc.dram_tensor("out", (TPAD, D), f32, kind="ExternalOutput")

    with tile.TileContext(nc) as tc, ExitStack() as ctx:
        ep = ctx.enter_context
        const_p = ep(tc.tile_pool(name="const", bufs=1))
        kvd = ep(tc.tile_pool(name="kvd", bufs=1, space="DRAM"))

        ident32 = const_p.tile([128, 128], f32)
        nc.sync.dma_start(ident32[:], dr["ident32"].ap())
        ident16 = const_p.tile([128, 128], f16)
        nc.sync.dma_start(ident16[:], dr["ident16"].ap())
        ones16 = const_p.tile([1, 128], f16)
        nc.sync.dma_start(ones16[:], dr["ones16"].ap())
        idx_sb = const_p.tile([128, MW], mybir.dt.int32)
        nc.sync.dma_start(idx_sb[:], dr["idxs"].ap())
        mask_sb = const_p.tile([128, MW], f32)
        nc.sync.dma_start(mask_sb[:], dr["masks"].ap())

        def ldw(name, chunks, ncol):
            t = const_p.tile([128, L, chunks, ncol], f16, tag="w_" + name)
            nc.sync.dma_start(
                t[:], dr[name].ap().rearrange("l (c p) n -> p l c n", p=128))
            return t
        wq_sb = ldw("wqT", 2, D)
        wkv_sb = ldw("wkvT", 2, 2 * D)
        wo_sb = ldw("woT", 2, D)
        w1_sb = ldw("w1T", 2, 2 * D)
        w2_sb = ldw("w2T", 4, D)
        brow = {}
        for name, ncol in (("bq", D), ("bkv", 2 * D), ("bo", D),
                           ("b1", 2 * D), ("b2", D)):
            t = const_p.tile([1, L, ncol], f16, tag="b_" + name)
            nc.sync.dma_start(t[:], dr[name].ap().rearrange("l o n -> o l n"))
            brow[name] = t
        lnbc = {}
        for name in ("ln1g", "ln1b", "ln2g", "ln2b"):
            t = const_p.tile([128, L, D], f32, tag="ln_" + name)
            nc.sync.dma_start(
                t[:], dr[name].ap().rearrange("l o n -> o l n")
                .broadcast_to([128, L, D]))
            lnbc[name] = t
        eps_sb = const_p.tile([128, 1], f32)
        nc.vector.memset(eps_sb[:], float(EPS))

        # combined K/V table: row = [k_l0 | v_l0 | k_l1 | v_l1], 2KB fp16
        kvtab = kvd.tile([NPAD, 2 * L * D], f16)

        # ---------- phase 0: K/V tables for both layers ----------
        with tc.tile_pool(name="p0sp", bufs=2) as p0sp, \
             tc.tile_pool(name="p0st", bufs=4) as p0st, \
             tc.tile_pool(name="p0ps", bufs=4, space="PSUM") as p0ps:
            CH = 8192
            off = 0
            while off < NPAD:
                w = min(CH, NPAD - off)
                sp0 = p0sp.tile([128, w], f16, tag="sp0")
                nc.sync.dma_start(sp0[:], dr["spatialT"].ap()[0:128, off:off + w])
                sp1 = p0sp.tile([128, w], f16, tag="sp1")
                nc.sync.dma_start(sp1[:], dr["spatialT"].ap()[128:256, off:off + w])
                for blk in range(w // 128):
                    st = p0st.tile([128, 2 * L * D], f16, tag="kvst")
                    for l in range(L):
                        ps = p0ps.tile([128, 2 * D], f32, tag="kvps")
                        nc.tensor.matmul(ps[:], sp0[:, bass.ts(blk, 128)],
                                         wkv_sb[:, l, 0, :], start=True, stop=False)
                        nc.tensor.matmul(ps[:], sp1[:, bass.ts(blk, 128)],
                                         wkv_sb[:, l, 1, :], start=False, stop=False)
                        nc.tensor.matmul(ps[:], ones16[:], brow["bkv"][:, l, :],
                                         start=False, stop=True)
                        nc.vector.tensor_copy(
                            st[:, 2 * D * l: 2 * D * (l + 1)], ps[:])
                    nc.sync.dma_start(
                        kvtab[off + blk * 128: off + (blk + 1) * 128, :], st[:])
                off += w

        # ---------- per-block processing, both layers ----------
        with tc.tile_pool(name="prod", bufs=1) as prodp, \
             tc.tile_pool(name="small", bufs=2) as smallp, \
             tc.tile_pool(name="mid", bufs=2) as midp, \
             tc.tile_pool(name="lnp", bufs=1) as lnp, \
             tc.tile_pool(name="psmm", bufs=3, space="PSUM") as psmm, \
             tc.tile_pool(name="pstp", bufs=4, space="PSUM") as pstp:

            def transpose_to_f16(src_ap, chunks, dst_tag):
                dst = midp.tile([128, chunks, 128], f16, tag=dst_tag)
                ident = ident32 if src_ap.dtype == f32 else ident16
                for cix in range(chunks):
                    tp = pstp.tile([128, 128], src_ap.dtype, tag="tp")
                    nc.tensor.transpose(tp[:], src_ap[:, bass.ts(cix, 128)],
                                        ident[:])
                    nc.vector.tensor_copy(dst[:, cix, :], tp[:])
                return dst

            def layernorm(src_ap, add_psum, gbc, bbc, out_tag):
                xr = lnp.tile([128, D], f32, tag="ln_xr")
                nc.vector.tensor_tensor(xr[:], src_ap, add_psum,
                                        op=mybir.AluOpType.add)
                sm = smallp.tile([128, 1], f32, tag="ln_sm")
                nc.vector.tensor_reduce(sm[:], xr[:],
                                        axis=mybir.AxisListType.X,
                                        op=mybir.AluOpType.add)
                mu = smallp.tile([128, 1], f32, tag="ln_mu")
                nc.vector.tensor_scalar_mul(mu[:], sm[:], 1.0 / D)
                xc = lnp.tile([128, D], f32, tag="ln_xc")
                nc.vector.tensor_scalar(xc[:], xr[:], scalar1=mu[:],
                                        scalar2=None,
                                        op0=mybir.AluOpType.subtract)
                sq = lnp.tile([128, D], f32, tag="ln_sq")
                nc.vector.tensor_tensor(sq[:], xc[:], xc[:],
                                        op=mybir.AluOpType.mult)
                vs = smallp.tile([128, 1], f32, tag="ln_vs")
                nc.vector.tensor_reduce(vs[:], sq[:],
                                        axis=mybir.AxisListType.X,
                                        op=mybir.AluOpType.add)
                var = smallp.tile([128, 1], f32, tag="ln_var")
                nc.vector.tensor_scalar_mul(var[:], vs[:], 1.0 / D)
                sd = smallp.tile([128, 1], f32, tag="ln_sd")
                nc.scalar.activation(sd[:], var[:],
                                     mybir.ActivationFunctionType.Sqrt,
                                     bias=eps_sb[:])
                rstd = smallp.tile([128, 1], f32, tag="ln_rs")
                nc.vector.reciprocal(rstd[:], sd[:])
                t1 = lnp.tile([128, D], f32, tag="ln_t1")
                nc.vector.tensor_scalar(t1[:], xc[:], scalar1=rstd[:],
                                        scalar2=None,
                                        op0=mybir.AluOpType.mult)
                t2 = lnp.tile([128, D], f32, tag="ln_t2")
                nc.vector.tensor_tensor(t2[:], t1[:], gbc,
                                        op=mybir.AluOpType.mult)
                xo = lnp.tile([128, D], f32, tag=out_tag)
                nc.vector.tensor_tensor(xo[:], t2[:], bbc,
                                        op=mybir.AluOpType.add)
                return xo

            moffs = np.concatenate([[0], np.cumsum(kblocks)]).astype(int)

            def do_block(b, pool, tag):
                K = kblocks[b]
                mo = int(moffs[b])
                # one gather covers K and V for BOTH layers (2KB rows)
                kvg = pool.tile([128, K, 2 * L * D], f16, tag=tag)
                for k in range(K):
                    nc.gpsimd.indirect_dma_start(
                        out=kvg[:, k, :], out_offset=None,
                        in_=kvtab[:],
                        in_offset=bass.IndirectOffsetOnAxis(
                            ap=idx_sb[:, mo + k:mo + k + 1], axis=0))

                xblk_t = midp.tile([128, D], f32, tag="xblk")
                nc.sync.dma_start(xblk_t[:],
                                  dr["x0"].ap()[b * 128:(b + 1) * 128, :])
                xcur = xblk_t[:]

                for l in range(L):
                    # q projection
                    xT = transpose_to_f16(xcur, 2, "xT")
                    qp = psmm.tile([128, D], f32, tag="mm")
                    nc.tensor.matmul(qp[:], xT[:, 0, :], wq_sb[:, l, 0, :],
                                     start=True, stop=False)
                    nc.tensor.matmul(qp[:], xT[:, 1, :], wq_sb[:, l, 1, :],
                                     start=False, stop=False)
                    nc.tensor.matmul(qp[:], ones16[:], brow["bq"][:, l, :],
                                     start=False, stop=True)
                    qh = smallp.tile([128, D], f16, tag="qh")
                    nc.vector.tensor_copy(qh[:], qp[:])

                    # scores = sum_d q*k -> [128, H, K] (two head-pairs)
                    k_ap = kvg[:, :, 2 * D * l: 2 * D * l + D].rearrange(
                        "p s (h d) -> p h s d", h=H)
                    q_ap = (qh[:].rearrange("p (h d) -> p h d", h=H)
                            .unsqueeze(2).broadcast_to([128, H, K, DH]))
                    scores = smallp.tile([128, H, K], f32, tag="scores")
                    prod = prodp.tile([128, H, K, DH], f16, tag="prod")
                    nc.vector.tensor_tensor(prod[:], k_ap, q_ap,
                                            op=mybir.AluOpType.mult)
                    nc.vector.tensor_reduce(scores[:], prod[:],
                                            axis=mybir.AxisListType.X,
                                            op=mybir.AluOpType.add)
                    masked = smallp.tile([128, H, K], f32, tag="masked")
                    m_ap = (mask_sb[:, mo:mo + K].unsqueeze(1)
                            .broadcast_to([128, H, K]))
                    nc.vector.tensor_tensor(masked[:], scores[:], m_ap,
                                            op=mybir.AluOpType.add)
                    ex = smallp.tile([128, H, K], f32, tag="ex")
                    nc.scalar.activation(ex[:], masked[:],
                                         mybir.ActivationFunctionType.Exp,
                                         scale=float(SCALE))
                    denom = smallp.tile([128, H], f32, tag="denom")
                    nc.vector.tensor_reduce(denom[:], ex[:],
                                            axis=mybir.AxisListType.X,
                                            op=mybir.AluOpType.add)
                    rden = smallp.tile([128, H], f32, tag="rden")
                    nc.vector.reciprocal(rden[:], denom[:])
                    alpha = smallp.tile([128, H, K], f16, tag="alpha")
                    r_ap = rden[:].unsqueeze(2).broadcast_to([128, H, K])
                    nc.vector.tensor_tensor(alpha[:], ex[:], r_ap,
                                            op=mybir.AluOpType.mult)

                    # AV: sum_s alpha*v -> [128, H, DH]
                    v_ap = kvg[:, :, 2 * D * l + D: 2 * D * (l + 1)].rearrange(
                        "p s (h d) -> p h d s", h=H)
                    a_ap = alpha[:].unsqueeze(2).broadcast_to([128, H, DH, K])
                    ao = smallp.tile([128, H, DH], f32, tag="ao")
                    prod2 = prodp.tile([128, H, DH, K], f16, tag="prod")
                    nc.vector.tensor_tensor(prod2[:], v_ap, a_ap,
                                            op=mybir.AluOpType.mult)
                    nc.vector.tensor_reduce(ao[:], prod2[:],
                                            axis=mybir.AxisListType.X,
                                            op=mybir.AluOpType.add)

                    # out projection
                    aoT = transpose_to_f16(
                        ao[:].rearrange("p h d -> p (h d)"), 2, "aoT")
                    pso = psmm.tile([128, D], f32, tag="mm")
                    nc.tensor.matmul(pso[:], aoT[:, 0, :], wo_sb[:, l, 0, :],
                                     start=True, stop=False)
                    nc.tensor.matmul(pso[:], aoT[:, 1, :], wo_sb[:, l, 1, :],
                                     start=False, stop=False)
                    nc.tensor.matmul(pso[:], ones16[:], brow["bo"][:, l, :],
                                     start=False, stop=True)

                    x1 = layernorm(xcur, pso[:], lnbc["ln1g"][:, l, :],
                                   lnbc["ln1b"][:, l, :], "x1_%d" % l)

                    # FFN
                    x1T = transpose_to_f16(x1[:], 2, "x1T")
                    psh = psmm.tile([128, 2 * D], f32, tag="mm")
                    nc.tensor.matmul(psh[:], x1T[:, 0, :], w1_sb[:, l, 0, :],
                                     start=True, stop=False)
                    nc.tensor.matmul(psh[:], x1T[:, 1, :], w1_sb[:, l, 1, :],
                                     start=False, stop=False)
                    nc.tensor.matmul(psh[:], ones16[:], brow["b1"][:, l, :],
                                     start=False, stop=True)
                    hh = midp.tile([128, 2 * D], f16, tag="hh")
                    nc.scalar.activation(hh[:], psh[:],
                                         mybir.ActivationFunctionType.Gelu)
                    hT = transpose_to_f16(hh[:], 4, "hT")
                    psy = psmm.tile([128, D], f32, tag="mm")
                    for cix in range(4):
                        nc.tensor.matmul(psy[:], hT[:, cix, :],
                                         w2_sb[:, l, cix, :],
                                         start=(cix == 0), stop=False)
                    nc.tensor.matmul(psy[:], ones16[:], brow["b2"][:, l, :],
                                     start=False, stop=True)

                    x2 = layernorm(x1[:], psy[:], lnbc["ln2g"][:, l, :],
                                   lnbc["ln2b"][:, l, :], "x2_%d" % l)
                    xcur = x2[:]

                nc.sync.dma_start(out_dram.ap()[b * 128:(b + 1) * 128, :],
                                  xcur)

            # low-K blocks get a double-buffered gather pool (overlap the
            # next block's gather with this block's compute); the high-K
            # tail runs afterward in its own single-buffer scope.
            KSPLIT = 24
            small_blocks = [b for b in range(NBLK) if kblocks[b] <= KSPLIT]
            big_blocks = [b for b in range(NBLK) if kblocks[b] > KSPLIT]
            if small_blocks:
                with tc.tile_pool(name="kvgA", bufs=2) as kvA:
                    for b in small_blocks:
                        do_block(b, kvA, "kvgA")
            if big_blocks:
                with tc.tile_pool(name="kvgB", bufs=1) as kvB:
                    for b in big_blocks:
                        do_block(b, kvB, "kvgB")

    nc.compile()
    return nc


def kernel(**inputs) -> np.ndarray:
    in_maps, tgt_ids, kblocks = _host_prep(inputs)
    if kblocks not in _prog_cache:
        _prog_cache[kblocks] = _build_program(kblocks)
    nc = _prog_cache[kblocks]
    res = bass_utils.run_bass_kernel_spmd(nc, in_maps,
                                          core_ids=list(range(NCORES)))
    out = np.zeros((N, D), np.float32)
    for c in range(NCORES):
        o = res.results[c]["out"]
        tg = tgt_ids[c]
        valid = tg >= 0
        out[tg[valid]] = o[valid]
    return out
